# revision 20
# baseline (speedup 1.0000x reference)
"""Trainium2 Bass kernel: DigitCapsules dynamic routing (CapsNet).

Problem: x [B=128, R=1152, I=64], W [R, C=32, O=32, I=64]
  u_hat = einsum('rcoi,bri->brco', W, x)
  3 routing iterations (softmax over C, weighted sum over R, squash)
  output v [B, C, O]

Sharding: R split across 8 cores (144 routes each), W never replicated.
Per routing iteration u_hat is recomputed on the PE from SBUF-resident x
and streamed W (u_hat is 75 MB/core - too big for SBUF, and HBM round
trips are slower than recompute).  The per-route routing contractions
(agreement b += u.v and weighted sum s += c*u) run on DVE/GPSIMD reading
u_hat straight out of PSUM.  Cross-core reduction of s via AllReduce.
"""

import numpy as np

import concourse.bass as bass
import concourse.bacc as bacc
import concourse.mybir as mybir
import concourse.tile as tile
from concourse.bass_utils import run_bass_kernel_spmd

B, R, C, O, I = 128, 1152, 32, 32, 64
NCORES = 8
RL = R // NCORES          # 144 routes per core
R2 = RL // 2              # 72 route pairs (2 routes share one 128-part tile)
CO = C * O                # 1024
G = 2                     # route-pairs per routing group -> 4 u tiles = 8 PSUM banks
NGROUPS = R2 // G
EPS = 1e-8
f32 = mybir.dt.float32
f32r = mybir.dt.float32r
bf16 = mybir.dt.bfloat16
AX = mybir.AxisListType
ALU = mybir.AluOpType
ACTF = mybir.ActivationFunctionType


def _bcast_inner(ap, n):
    """[P, ...] -> [P, ..., n] broadcast (step 0) along a new inner axis."""
    return bass.AP(tensor=ap.tensor, offset=ap.offset, ap=[*ap.ap, [0, n]])


def _as3d(ap):
    """[P, CO] view -> [P, C, O]."""
    return ap.rearrange("p (c o) -> p c o", o=O)


def _pe_absorb(nc, psum_ap, src_ap):
    """Tiny 1x1 matmul: absorbs one cross-engine wait into PE program order.

    The self-loading f32r Matmult has a single sync-wait slot in its ISA
    encoding; any matmul with >=2 cross-engine deps fails codegen.  A dummy
    matmul takes one dep; the real matmul then inherits it for free via
    same-engine ordering."""
    nc.tensor.matmul(
        psum_ap[0:1, 0:1],
        lhsT=src_ap,
        rhs=src_ap,
        start=True,
        stop=True,
        skip_group_check=True,
    )


def _allreduce_squash(nc, tc, pools, tag, s_sb, v_sb, scale):
    """v_sb = squash(scale * allreduce_sum(s_sb)) ; all [B, CO] f32 SBUF."""
    sm = pools["small"]
    big = pools["big"]

    cc_in = nc.dram_tensor(f"cc_in_{tag}", [B, CO], f32, kind="Internal")
    cc_out = nc.dram_tensor(
        f"cc_out_{tag}", [B, CO], f32, kind="Internal", addr_space="Shared"
    )
    nc.gpsimd.dma_start(out=cc_in[:], in_=s_sb[:])
    nc.gpsimd.collective_compute(
        "AllReduce",
        ALU.add,
        replica_groups=[list(range(NCORES))],
        ins=[cc_in[:].opt()],
        outs=[cc_out[:].opt()],
    )
    st = big.tile([B, CO], f32, tag="st")
    nc.gpsimd.dma_start(out=st[:], in_=cc_out[:])

    if scale != 1.0:
        nc.vector.tensor_scalar_mul(st, st, float(scale))
    # n2[b,c] = sum_o st^2
    sq = big.tile([B, CO], f32, tag="sq")
    nc.scalar.activation(sq, st, ACTF.Square)
    n2 = sm.tile([B, C], f32, tag="n2")
    nc.vector.tensor_reduce(n2, _as3d(sq[:]), axis=AX.X, op=ALU.add)
    # factor = n2 / ((1 + n2) * (sqrt(n2) + eps))
    sr = sm.tile([B, C], f32, tag="sr")
    nc.scalar.activation(sr, n2, ACTF.Sqrt)
    a1 = sm.tile([B, C], f32, tag="a1")
    nc.vector.tensor_scalar_add(a1, n2, 1.0)
    a2 = sm.tile([B, C], f32, tag="a2")
    nc.vector.tensor_scalar_add(a2, sr, float(EPS))
    nc.vector.tensor_mul(a1, a1, a2)
    rc = sm.tile([B, C], f32, tag="rc")
    nc.vector.reciprocal(rc, a1)
    fac = sm.tile([B, C], f32, tag="fac")
    nc.vector.tensor_mul(fac, n2, rc)
    nc.vector.tensor_tensor(
        out=_as3d(v_sb[:]), in0=_as3d(st[:]), in1=_bcast_inner(fac[:], O), op=ALU.mult
    )


def _routing_pass(nc, tc, pools, x_sb, w_t, v_sb, b1_sb, s_sb, first, psum, wpool):
    """One routing iteration: recompute u_hat per route; update logits,
    softmax over C, accumulate s = sum_r c*u.  first=True means prior
    logits are zero (iteration 1).

    The agreement chain (h = u*v, reduce over O) runs in bf16 so the DVE
    hits its 2x packed mode; u escapes PSUM once via an ACT bf16 copy.
    The s accumulation chain stays f32."""
    big = pools["big"]
    sm = pools["small"]
    b16 = pools["big16"]

    nc.gpsimd.memset(s_sb[:], 0.0)
    v16 = b16.tile([B, CO], bf16, tag="v16")
    nc.vector.tensor_copy(v16, v_sb)

    for g in range(NGROUPS):
        us = []
        u16s = []
        for j2 in range(G):
            r2 = g * G + j2
            w = wpool.tile([128, CO], f32r, tag="w")
            nc.sync.dma_start(
                out=w[:],
                in_=w_t[2 * r2 : 2 * r2 + 2]
                .rearrange("t i n -> (t i) n")
                .bitcast(f32r),
            )
            for half in (0, 1):
                u = psum.tile([B, CO], f32, tag="u")
                for n in (0, 1):
                    nc.tensor.matmul(
                        u[:, 512 * n : 512 * n + 512],
                        lhsT=x_sb[64 * half : 64 * half + 64, r2, :],
                        rhs=w[64 * half : 64 * half + 64, 512 * n : 512 * n + 512],
                        start=True,
                        stop=True,
                    )
                us.append(u)
                u16 = b16.tile([B, CO], bf16, tag="u16")
                nc.scalar.activation(u16, u, ACTF.Copy)
                u16s.append(u16)
        r0 = g * G * 2
        nr = 2 * G
        # agreement: bu[b, r, c] = sum_o u[b, (c,o)] * v[b, (c,o)]
        bu = sm.tile([B, nr, C], f32, tag="bu")
        for j, u16 in enumerate(u16s):
            h = b16.tile([B, CO], bf16, tag="h")
            nc.vector.tensor_mul(h, u16, v16)
            dst = b1_sb[:, r0 + j, :] if first else bu[:, j, :]
            nc.vector.tensor_reduce(dst, _as3d(h[:]), axis=AX.X, op=ALU.add)
        if first:
            lg = b1_sb[:, r0 : r0 + nr, :]
        else:
            lg = sm.tile([B, nr, C], f32, tag="lg")
            nc.vector.tensor_add(lg, b1_sb[:, r0 : r0 + nr, :], bu)
        # softmax over C for each (b, r)
        mx = sm.tile([B, nr], f32, tag="mx")
        nc.vector.tensor_reduce(mx, lg, axis=AX.X, op=ALU.max)
        ex = sm.tile([B, nr, C], f32, tag="ex")
        nc.vector.tensor_tensor(
            out=ex[:], in0=lg, in1=_bcast_inner(mx[:], C), op=ALU.subtract
        )
        ce = sm.tile([B, nr, C], f32, tag="ce")
        nc.scalar.activation(ce, ex, ACTF.Exp)
        ssum = sm.tile([B, nr], f32, tag="ssum")
        nc.vector.tensor_reduce(ssum, ce, axis=AX.X, op=ALU.add)
        rc = sm.tile([B, nr], f32, tag="rcs")
        nc.vector.reciprocal(rc, ssum)
        nc.vector.tensor_tensor(
            out=ce[:], in0=ce[:], in1=_bcast_inner(rc[:], C), op=ALU.mult
        )
        # s += c * u   (product on DVE, accumulate on GPSIMD)
        for j, u in enumerate(us):
            t = big.tile([B, CO], f32, tag="t")
            nc.vector.tensor_tensor(
                out=_as3d(t[:]),
                in0=_as3d(u[:]),
                in1=_bcast_inner(ce[:, j, :], O),
                op=ALU.mult,
            )
            nc.gpsimd.tensor_add(s_sb, s_sb, t)


def build_kernel(reps=1):
    """reps>1 repeats the whole computation in one NEFF (timing only)."""
    nc = bacc.Bacc("TRN2", num_devices=NCORES, target_bir_lowering=False)
    # per-core inputs, host pre-transposed:
    #   x_t[r, i, b]  (local routes)      w_t[r, i, c*o]
    x_t = nc.dram_tensor("x_t", [RL, I, B], f32, kind="ExternalInput")
    w_t = nc.dram_tensor("w_t", [RL, I, CO], f32, kind="ExternalInput")
    v_out = nc.dram_tensor("v_out", [B, CO], f32, kind="ExternalOutput")

    with tile.TileContext(nc) as tc:
        singles = tc.alloc_tile_pool(name="singles", bufs=1)
        big = tc.alloc_tile_pool(name="big", bufs=3)
        small = tc.alloc_tile_pool(name="small", bufs=3)
        big16 = tc.alloc_tile_pool(name="big16", bufs=4)
        wpool = tc.alloc_tile_pool(name="wpool", bufs=8)
        pools = {"big": big, "small": small, "big16": big16}

        # x resident in SBUF: partitions (parity, i), free (r2, b)
        x_sb = singles.tile([128, R2, B], f32r, tag="x")
        xr = x_t[:].rearrange("(r2 two) i b -> (two i) r2 b", two=2).bitcast(f32r)
        nc.sync.dma_start(out=x_sb[:, :, :], in_=xr)

        v_sb = singles.tile([B, CO], f32, tag="v")
        s_sb = singles.tile([B, CO], f32, tag="s")
        b1_sb = singles.tile([B, RL, C], f32, tag="b1")

        for rep in range(reps):
            # ---- pass A: s0 = sum_r u_r (uniform c), K=128 over (2 routes x I)
            with tc.tile_pool(name=f"psA{rep}", bufs=1, space="PSUM") as psA:
                s0 = psA.tile([B, CO], f32, tag="s0")
                for r2 in range(R2):
                    w = wpool.tile([128, CO], f32r, tag="w")
                    nc.sync.dma_start(
                        out=w[:],
                        in_=w_t[2 * r2 : 2 * r2 + 2]
                        .rearrange("t i n -> (t i) n")
                        .bitcast(f32r),
                    )
                    for n in (0, 1):
                        nc.tensor.matmul(
                            s0[:, 512 * n : 512 * n + 512],
                            lhsT=x_sb[:, r2, :],
                            rhs=w[:, 512 * n : 512 * n + 512],
                            start=(r2 == 0),
                            stop=(r2 == R2 - 1),
                            skip_group_check=True,
                        )
                nc.vector.tensor_copy(s_sb, s0)
            _allreduce_squash(nc, tc, pools, f"{rep}_0", s_sb, v_sb, 1.0 / C)

            # ---- passes B, C: full routing iterations
            with tc.tile_pool(name=f"psB{rep}", bufs=4, space="PSUM") as psB:
                _routing_pass(
                    nc, tc, pools, x_sb, w_t, v_sb, b1_sb, s_sb, True, psB, wpool
                )
                _allreduce_squash(nc, tc, pools, f"{rep}_1", s_sb, v_sb, 1.0)
                _routing_pass(
                    nc, tc, pools, x_sb, w_t, v_sb, b1_sb, s_sb, False, psB, wpool
                )
                _allreduce_squash(nc, tc, pools, f"{rep}_2", s_sb, v_sb, 1.0)

        nc.sync.dma_start(out=v_out[:], in_=v_sb[:])

        for p in (wpool, big16, small, big, singles):
            p.release()
    nc.finalize()  # Bacc.compile(): splits multi-wait instructions, alloc regs
    return nc


_NC_CACHE = None


def _get_nc():
    global _NC_CACHE
    if _NC_CACHE is None:
        _NC_CACHE = build_kernel()
    return _NC_CACHE


def _make_in_maps(x, W):
    in_maps = []
    for k in range(NCORES):
        rs = slice(k * RL, (k + 1) * RL)
        x_t = np.ascontiguousarray(np.transpose(x[:, rs, :], (1, 2, 0)))  # [RL, I, B]
        w_t = np.ascontiguousarray(
            np.transpose(W[rs].reshape(RL, CO, I), (0, 2, 1))
        )  # [RL, I, CO]
        in_maps.append({"x_t": x_t.astype(np.float32), "w_t": w_t.astype(np.float32)})
    return in_maps


def run(x, W, **run_kwargs):
    nc = _get_nc()
    res = run_bass_kernel_spmd(
        nc, _make_in_maps(x, W), core_ids=list(range(NCORES)), **run_kwargs
    )
    v = res.results[0]["v_out"].reshape(B, C, O)
    return v, res


class _Runner:
    """Persistent jitted executor (mirrors bass2jax.run_bass_via_pjrt's
    multi-core path but caches the jitted callable across calls)."""

    def __init__(self, nc):
        import jax
        from jax.sharding import Mesh, PartitionSpec
        from jax.experimental.shard_map import shard_map
        from concourse import bass2jax

        bass2jax.install_neuronx_cc_hook()
        self.jax = jax
        self.nc = nc
        pname = nc.partition_id_tensor.name if nc.partition_id_tensor else None
        in_names, out_names, out_avals, zero_outs = [], [], [], []
        for alloc in nc.m.functions[0].allocations:
            if not isinstance(alloc, mybir.MemoryLocationSet):
                continue
            name = alloc.memorylocations[0].name
            if alloc.kind == "ExternalInput":
                if name != pname:
                    in_names.append(name)
            elif alloc.kind == "ExternalOutput":
                shape = tuple(alloc.tensor_shape)
                dtype = mybir.dt.np(alloc.dtype)
                out_names.append(name)
                out_avals.append(jax.core.ShapedArray(shape, dtype))
                zero_outs.append(np.zeros(shape, dtype))
        self.in_names, self.out_names = list(in_names), out_names
        self.out_avals, self.zero_outs = out_avals, zero_outs
        n_params = len(in_names)
        all_in = in_names + out_names + ([pname] if pname else [])

        def _body(*args):
            operands = list(args)
            if pname is not None:
                operands.append(bass2jax.partition_id_tensor())
            return tuple(
                bass2jax._bass_exec_p.bind(
                    *operands,
                    out_avals=tuple(out_avals),
                    in_names=tuple(all_in),
                    out_names=tuple(out_names),
                    lowering_input_output_aliases=(),
                    sim_require_finite=True,
                    sim_require_nnan=True,
                    nc=nc,
                )
            )

        devices = jax.devices()[:NCORES]
        self.mesh = Mesh(np.asarray(devices), ("core",))
        n_outs = len(out_names)
        self.fn = jax.jit(
            shard_map(
                _body,
                mesh=self.mesh,
                in_specs=(PartitionSpec("core"),) * (n_params + n_outs),
                out_specs=(PartitionSpec("core"),) * n_outs,
                check_rep=False,
            ),
            donate_argnums=tuple(range(n_params, n_params + n_outs)),
            keep_unused=True,
        )

    def concat_inputs(self, in_maps):
        return [
            np.concatenate([np.asarray(m[name]) for m in in_maps], axis=0)
            for name in self.in_names
        ]

    def zeros(self):
        return [
            np.zeros((NCORES * z.shape[0], *z.shape[1:]), z.dtype)
            for z in self.zero_outs
        ]

    def run_arrays(self, concat_in):
        outs = self.fn(*concat_in, *self.zeros())
        return outs

    def run_numpy(self, in_maps):
        outs = self.run_arrays(self.concat_inputs(in_maps))
        res = []
        for c in range(NCORES):
            res.append(
                {
                    name: np.asarray(outs[i]).reshape(
                        NCORES, *self.out_avals[i].shape
                    )[c]
                    for i, name in enumerate(self.out_names)
                }
            )
        return res


_RUNNER = None


def _get_runner():
    global _RUNNER
    if _RUNNER is None:
        _RUNNER = _Runner(_get_nc())
    return _RUNNER


def kernel(x, W):
    r = _get_runner()
    res = r.run_numpy(_make_in_maps(np.asarray(x), np.asarray(W)))
    return res[0]["v_out"].reshape(B, C, O).astype(np.float32)


def bench(x, W, iters=10, reps=1, runner=None):
    """Steady-state per-call wall times for a reps-repeated kernel."""
    import time as _time

    import jax

    if runner is None:
        runner = _Runner(_get_nc() if reps == 1 else build_kernel(reps))
    r = runner
    concat = r.concat_inputs(_make_in_maps(np.asarray(x), np.asarray(W)))
    from jax.sharding import NamedSharding, PartitionSpec

    sh = NamedSharding(r.mesh, PartitionSpec("core"))
    dev_in = [jax.device_put(a, sh) for a in concat]
    out = r.run_arrays(dev_in)  # warm
    jax.block_until_ready(out)
    times = []
    for _ in range(iters):
        t0 = _time.perf_counter()
        out = r.run_arrays(dev_in)
        jax.block_until_ready(out)
        times.append(_time.perf_counter() - t0)
    v = np.asarray(out[0]).reshape(NCORES, B, CO)[0].reshape(B, C, O)
    return v, times


if __name__ == "__main__":
    rng = np.random.default_rng(0)
    x = rng.standard_normal((B, R, I), dtype=np.float32)
    W = (0.01 * rng.standard_normal((R, C, O, I))).astype(np.float32)
    v, _ = run(x, W)
    print(v.shape, float(np.abs(v).max()))
